# revision 23
# baseline (speedup 1.0000x reference)
"""Trainium2 Bass kernel for DiscreteGCNLayer.

Computation (per batch b):
    dw      = ternary_quantize(weight, s=0.01)            # [256, 256]
    support = x[b] @ dw                                   # [2048, 256]
    out[b]  = relu(adj[b] @ support + bias)               # [2048, 256]

Strategy: data-parallel over the batch dim (8 batches -> 8 NeuronCores),
weight/bias replicated.  Per core:
  stage 1: support[m, o] = sum_i x[m, i] dw[i, o]   (x tiles PE-transposed)
  stage 2: out[n, o] = relu(sum_m adj[n, m] support[m, o] + bias[o])
adj row-blocks stream from HBM; each 128x128 adj tile is PE-transposed
(fp32r transpose mode), 4 transposes share one PSUM bank so a single
[128, 512] copy moves them to SBUF (amortizes the PSUM access latency and
semaphore hops), then consumed as matmul lhsT.  All matmuls run as float32r
(full-rate fp32 on the PE for moving dims >= 256).  The bias add is folded
into the PSUM accumulation as a rank-1 matmul with a ones row; relu rides
the PSUM->SBUF eviction on the scalar engine.
"""

import os
import sys

import numpy as np

if "/opt/trn_rl_repo" not in sys.path:
    sys.path.insert(0, "/opt/trn_rl_repo")

B = 8
N = 2048
DIN = 256
DOUT = 256
P = 128
NB = N // P  # 16 row blocks
MB = N // P  # 16 contraction chunks (stage 2)
IB = DIN // P  # 2 contraction chunks (stage 1)
SPARSITY = 0.01

_NC = None


def _build_nc():
    from contextlib import ExitStack

    import concourse.bass as bass
    import concourse.mybir as mybir
    import concourse.tile as tile
    from concourse import bacc
    from concourse.bass import ts
    from concourse.masks import make_identity

    F32 = mybir.dt.float32
    F32R = mybir.dt.float32r
    Alu = mybir.AluOpType

    nc = bacc.Bacc()
    # x/adj/bias feed fp32r matmuls; declaring them float32r (bit-identical
    # 4-byte layout, numpy-binds as float32) keeps the DMA cast-free while
    # satisfying the verifier's fp32r producer-dtype rule.
    x_d = nc.dram_tensor("x", [N, DIN], F32R, kind="ExternalInput")
    adj_d = nc.dram_tensor("adj", [N, N], F32R, kind="ExternalInput")
    w_d = nc.dram_tensor("weight", [DIN, DOUT], F32, kind="ExternalInput")
    b_d = nc.dram_tensor("bias", [DOUT], F32R, kind="ExternalInput")
    out_d = nc.dram_tensor("out", [N, DOUT], F32, kind="ExternalOutput")

    with tile.TileContext(nc) as tc, ExitStack() as ctx:
        singles = ctx.enter_context(tc.tile_pool(name="singles", bufs=1))
        arow_pool = ctx.enter_context(tc.tile_pool(name="arow", bufs=6))
        at_dve = ctx.enter_context(tc.tile_pool(name="at_dve", bufs=6))
        at_act = ctx.enter_context(tc.tile_pool(name="at_act", bufs=6))
        out_pool = ctx.enter_context(tc.tile_pool(name="outsb", bufs=16))
        psum_t = ctx.enter_context(tc.tile_pool(name="pt", bufs=6, space="PSUM"))
        psum_acc = ctx.enter_context(tc.tile_pool(name="pacc", bufs=2, space="PSUM"))

        # PE warm-up burst: the HAM clock gate needs ~3.4us of sustained PE
        # activity to lift the 1.2 GHz cold throttle.  The PE would otherwise
        # sit idle during the DMA-bound startup; these dummy transposes of a
        # junk tile (memset on DVE so nothing upstream gates them) spend that
        # window ramping, while the identity builds on GPSIMD in parallel.
        junk = singles.tile([P, P], F32)
        nc.vector.memset(junk, 1.0)
        for wu in range(5):
            wt = psum_t.tile([P, 4 * P], F32, tag="pt")
            for j in range(4):
                nc.tensor.transpose(wt[:, j * P : (j + 1) * P], junk, junk)

        ident_f32 = singles.tile([P, P], F32)
        make_identity(nc, ident_f32)
        ident = singles.tile([P, P], F32R)
        nc.vector.tensor_copy(ident, ident_f32)

        ones_f32 = singles.tile([1, P], F32)
        nc.vector.memset(ones_f32, 1.0)
        ones = singles.tile([1, P], F32R)
        nc.vector.tensor_copy(ones, ones_f32)
        # weight first: dw must be ready when the stage-1 matmuls start.
        # ternary-quantized weight: dw = ((w > s) - (w < -s)) * s
        w_sb = singles.tile([P, IB, DOUT], F32)
        # SWDGE queue: runs in parallel with the SP-queue x/adj stream.
        nc.gpsimd.dma_start(out=w_sb, in_=w_d[:].rearrange("(c p) o -> p c o", p=P))
        dw_sb = singles.tile([P, IB, DOUT], F32R)
        tpos = singles.tile([P, IB, DOUT], F32)
        tneg = singles.tile([P, IB, DOUT], F32)
        nc.vector.tensor_scalar(
            out=tpos, in0=w_sb, scalar1=SPARSITY, scalar2=SPARSITY,
            op0=Alu.is_gt, op1=Alu.mult,
        )
        nc.vector.tensor_scalar(
            out=tneg, in0=w_sb, scalar1=-SPARSITY, scalar2=SPARSITY,
            op0=Alu.is_lt, op1=Alu.mult,
        )
        nc.vector.tensor_sub(dw_sb, tpos, tneg)

        # x_sb[p, c, i] = x[128 c + p, i]  (x in natural [m, i] chunks).
        # Loaded as 8 two-chunk DMAs so stage-1 group g can start as soon as
        # its chunks land instead of waiting for the whole 2 MB transfer.
        x_sb = singles.tile([P, MB, DIN], F32R)
        x_r = x_d[:].rearrange("(c p) i -> p c i", p=P)
        for g in range(MB // 2):
            nc.sync.dma_start(
                out=x_sb[:, 2 * g : 2 * g + 2, :], in_=x_r[:, 2 * g : 2 * g + 2, :]
            )

        bias_sb = singles.tile([1, DOUT], F32R)
        nc.gpsimd.dma_start(out=bias_sb, in_=b_d[:].rearrange("(p o) -> p o", p=1))

        # Warm-up gates: let the PE observe the ident producer and the x DMA
        # early so steady-state transposes carry only their direct-producer
        # wait (walrus allows a single semaphore wait on 4-byte matmuls; Bacc
        # splits extras but each split costs a sync hop).
        gate1 = psum_t.tile([P, 4 * P], F32R, tag="pt")
        nc.tensor.transpose(gate1[:, 0:P], ident, ident)
        nc.tensor.transpose(gate1[:, P : 2 * P], x_sb[:, 0, 0:P], ident)

        # stage 1: support[m-chunk c][p, o] = sum_i x[128c+p, i] dw[i, o]
        # xT tiles via PE transpose, grouped 4 per PSUM bank (2 m-chunks).
        support = singles.tile([P, MB, DOUT], F32R)
        for g in range(MB // 2):
            ptg = psum_t.tile([P, 4 * P], F32R, tag="pt")
            for k in range(2):
                c = 2 * g + k
                for ib in range(IB):
                    j = 2 * k + ib
                    nc.tensor.transpose(
                        ptg[:, j * P : (j + 1) * P],
                        x_sb[:, c, ts(ib, P)],
                        ident,
                    )
            if g % 2 == 0:
                atg = at_dve.tile([P, 4 * P], F32R, tag="at_dve")
                nc.vector.tensor_copy(atg, ptg)
            else:
                atg = at_act.tile([P, 4 * P], F32R, tag="at_act")
                nc.scalar.copy(atg, ptg)
            for k in range(2):
                c = 2 * g + k
                spsum = psum_acc.tile([P, DOUT], F32, tag="acc")
                for ib in range(IB):
                    j = 2 * k + ib
                    nc.tensor.matmul(
                        spsum,
                        lhsT=atg[:, j * P : (j + 1) * P],
                        rhs=dw_sb[:, ib, :],
                        start=(ib == 0),
                        stop=(ib == IB - 1),
                    )
                if c % 2 == 0:
                    nc.vector.tensor_copy(support[:, c, :], spsum)
                else:
                    nc.scalar.copy(support[:, c, :], spsum)

        # stage 2: out[n-block nb] = relu(adj[nb rows] @ support + bias).
        # Software-pipelined emission: the transpose group for global group
        # G is emitted BEFORE the matmuls of group G-1, so the PE's static
        # instruction order interleaves them and the PSUM->SBUF copy latency
        # of group G hides behind the matmuls of G-1 (instead of the PE
        # head-of-line blocking on its own just-issued transpose's copy).
        GPN = MB // 4  # transpose groups per n-block
        arows = {}
        opsums = {}

        def emit_tgroup(G):
            nb, g = divmod(G, GPN)
            if g == 0:
                arow = arow_pool.tile([P, N], F32R, tag="arow", name=f"arow{nb}")
                nc.sync.dma_start(out=arow, in_=adj_d[ts(nb, P), :])
                arows[nb] = arow
                opsums[nb] = psum_acc.tile(
                    [P, DOUT], F32, tag="acc", name=f"opsum{nb}"
                )
            arow = arows[nb]
            ptg = psum_t.tile([P, 4 * P], F32R, tag="pt")
            for k in range(4):
                mb = 4 * g + k
                nc.tensor.transpose(
                    ptg[:, k * P : (k + 1) * P], arow[:, ts(mb, P)], ident
                )
            if g % 2 == 0:
                atg = at_dve.tile([P, 4 * P], F32R, tag="at_dve")
                nc.vector.tensor_copy(atg, ptg)
            else:
                atg = at_act.tile([P, 4 * P], F32R, tag="at_act")
                nc.scalar.copy(atg, ptg)
            return atg

        def emit_mgroup(G, atg):
            nb, g = divmod(G, GPN)
            opsum = opsums[nb]
            for k in range(4):
                mb = 4 * g + k
                nc.tensor.matmul(
                    opsum,
                    lhsT=atg[:, k * P : (k + 1) * P],
                    rhs=support[:, mb, :],
                    start=(mb == 0),
                    stop=False,
                )
            if g == GPN - 1:
                nc.tensor.matmul(
                    opsum, lhsT=ones, rhs=bias_sb, start=False, stop=True
                )
                out_sb = out_pool.tile([P, DOUT], F32)
                nc.scalar.activation(
                    out_sb, opsum, mybir.ActivationFunctionType.Relu
                )
                # out stores ride the ACT HWDGE queue so they never
                # head-of-line block the SP queue's adj row prefetch stream.
                nc.scalar.dma_start(out=out_d[ts(nb, P), :], in_=out_sb)

        NGROUPS = NB * GPN
        LOOKAHEAD = 4
        pending = [emit_tgroup(G) for G in range(LOOKAHEAD)]
        for G in range(LOOKAHEAD, NGROUPS):
            nxt = emit_tgroup(G)
            emit_mgroup(G - LOOKAHEAD, pending.pop(0))
            pending.append(nxt)
        for i, atg in enumerate(pending):
            emit_mgroup(NGROUPS - LOOKAHEAD + i, atg)

    nc.compile()
    return nc


def _get_nc():
    global _NC
    if _NC is None:
        _NC = _build_nc()
    return _NC


def kernel(x, adj, weight, bias, _trace=False):
    from concourse import bass_utils

    x = np.ascontiguousarray(np.asarray(x, dtype=np.float32))
    adj = np.ascontiguousarray(np.asarray(adj, dtype=np.float32))
    weight = np.ascontiguousarray(np.asarray(weight, dtype=np.float32))
    bias = np.ascontiguousarray(np.asarray(bias, dtype=np.float32))

    nc = _get_nc()
    in_maps = [
        {"x": x[b], "adj": adj[b], "weight": weight, "bias": bias}
        for b in range(B)
    ]
    res = bass_utils.run_bass_kernel_spmd(
        nc, in_maps, core_ids=list(range(B)), trace=_trace
    )
    out = np.stack([r["out"] for r in res.results], axis=0)
    if _trace:
        return out, res
    return out


# revision 25
# speedup vs baseline: 1433.4148x; 1433.4148x over previous
"""Trainium2 Bass kernel for DiscreteGCNLayer.

Computation (per batch b):
    dw      = ternary_quantize(weight, s=0.01)            # [256, 256]
    support = x[b] @ dw                                   # [2048, 256]
    out[b]  = relu(adj[b] @ support + bias)               # [2048, 256]

Strategy: data-parallel over the batch dim (8 batches -> 8 NeuronCores),
weight/bias replicated.  Per core:
  stage 1: support[m, o] = sum_i x[m, i] dw[i, o]   (x tiles PE-transposed)
  stage 2: out[n, o] = relu(sum_m adj[n, m] support[m, o] + bias[o])
adj row-blocks stream from HBM; each 128x128 adj tile is PE-transposed
(fp32r transpose mode), 4 transposes share one PSUM bank so a single
[128, 512] copy moves them to SBUF (amortizes the PSUM access latency and
semaphore hops), then consumed as matmul lhsT.  All matmuls run as float32r
(full-rate fp32 on the PE for moving dims >= 256).  The bias add is folded
into the PSUM accumulation as a rank-1 matmul with a ones row; relu rides
the PSUM->SBUF eviction on the scalar engine.
"""

import os
import sys

import numpy as np

if "/opt/trn_rl_repo" not in sys.path:
    sys.path.insert(0, "/opt/trn_rl_repo")

B = 8
N = 2048
DIN = 256
DOUT = 256
P = 128
NB = N // P  # 16 row blocks
MB = N // P  # 16 contraction chunks (stage 2)
IB = DIN // P  # 2 contraction chunks (stage 1)
SPARSITY = 0.01

_NC = None


def _build_nc():
    from contextlib import ExitStack

    import concourse.bass as bass
    import concourse.mybir as mybir
    import concourse.tile as tile
    from concourse import bacc
    from concourse.bass import ts
    from concourse.masks import make_identity

    F32 = mybir.dt.float32
    F32R = mybir.dt.float32r
    Alu = mybir.AluOpType

    nc = bacc.Bacc()
    # x/adj/bias feed fp32r matmuls; declaring them float32r (bit-identical
    # 4-byte layout, numpy-binds as float32) keeps the DMA cast-free while
    # satisfying the verifier's fp32r producer-dtype rule.
    x_d = nc.dram_tensor("x", [N, DIN], F32R, kind="ExternalInput")
    adj_d = nc.dram_tensor("adj", [N, N], F32R, kind="ExternalInput")
    w_d = nc.dram_tensor("weight", [DIN, DOUT], F32, kind="ExternalInput")
    b_d = nc.dram_tensor("bias", [DOUT], F32R, kind="ExternalInput")
    out_d = nc.dram_tensor("out", [N, DOUT], F32, kind="ExternalOutput")

    with tile.TileContext(nc) as tc, ExitStack() as ctx:
        singles = ctx.enter_context(tc.tile_pool(name="singles", bufs=1))
        arow_pool = ctx.enter_context(tc.tile_pool(name="arow", bufs=6))
        at_dve = ctx.enter_context(tc.tile_pool(name="at_dve", bufs=6))
        at_act = ctx.enter_context(tc.tile_pool(name="at_act", bufs=6))
        out_pool = ctx.enter_context(tc.tile_pool(name="outsb", bufs=16))
        psum_t = ctx.enter_context(tc.tile_pool(name="pt", bufs=6, space="PSUM"))
        psum_acc = ctx.enter_context(tc.tile_pool(name="pacc", bufs=2, space="PSUM"))

        # PE warm-up burst: the HAM clock gate needs ~3.4us of sustained PE
        # activity to lift the 1.2 GHz cold throttle.  The PE would otherwise
        # sit idle during the DMA-bound startup; these dummy transposes of a
        # junk tile (memset on DVE so nothing upstream gates them) spend that
        # window ramping, while the identity builds on GPSIMD in parallel.
        junk = singles.tile([P, P], F32)
        nc.vector.memset(junk, 1.0)
        for wu in range(5):
            wt = psum_t.tile([P, 4 * P], F32, tag="pt")
            for j in range(4):
                nc.tensor.transpose(wt[:, j * P : (j + 1) * P], junk, junk)

        ident_f32 = singles.tile([P, P], F32)
        make_identity(nc, ident_f32)
        ident = singles.tile([P, P], F32R)
        nc.vector.tensor_copy(ident, ident_f32)

        ones_f32 = singles.tile([1, P], F32)
        nc.vector.memset(ones_f32, 1.0)
        ones = singles.tile([1, P], F32R)
        nc.vector.tensor_copy(ones, ones_f32)
        # weight first: dw must be ready when the stage-1 matmuls start.
        # ternary-quantized weight: dw = ((w > s) - (w < -s)) * s
        w_sb = singles.tile([P, IB, DOUT], F32)
        # SWDGE queue: runs in parallel with the SP-queue x/adj stream.
        nc.gpsimd.dma_start(out=w_sb, in_=w_d[:].rearrange("(c p) o -> p c o", p=P))
        dw_sb = singles.tile([P, IB, DOUT], F32R)
        tpos = singles.tile([P, IB, DOUT], F32)
        tneg = singles.tile([P, IB, DOUT], F32)
        nc.vector.tensor_scalar(
            out=tpos, in0=w_sb, scalar1=SPARSITY, scalar2=SPARSITY,
            op0=Alu.is_gt, op1=Alu.mult,
        )
        nc.vector.tensor_scalar(
            out=tneg, in0=w_sb, scalar1=-SPARSITY, scalar2=SPARSITY,
            op0=Alu.is_lt, op1=Alu.mult,
        )
        nc.vector.tensor_sub(dw_sb, tpos, tneg)

        # x_sb[p, c, i] = x[128 c + p, i]  (x in natural [m, i] chunks).
        # Loaded as 8 two-chunk DMAs so stage-1 group g can start as soon as
        # its chunks land instead of waiting for the whole 2 MB transfer.
        x_sb = singles.tile([P, MB, DIN], F32R)
        x_r = x_d[:].rearrange("(c p) i -> p c i", p=P)
        for g in range(MB // 2):
            nc.sync.dma_start(
                out=x_sb[:, 2 * g : 2 * g + 2, :], in_=x_r[:, 2 * g : 2 * g + 2, :]
            )

        bias_sb = singles.tile([1, DOUT], F32R)
        nc.gpsimd.dma_start(out=bias_sb, in_=b_d[:].rearrange("(p o) -> p o", p=1))

        # Warm-up gates: let the PE observe the ident producer and the x DMA
        # early so steady-state transposes carry only their direct-producer
        # wait (walrus allows a single semaphore wait on 4-byte matmuls; Bacc
        # splits extras but each split costs a sync hop).
        gate1 = psum_t.tile([P, 4 * P], F32R, tag="pt")
        nc.tensor.transpose(gate1[:, 0:P], ident, ident)
        nc.tensor.transpose(gate1[:, P : 2 * P], x_sb[:, 0, 0:P], ident)

        # stage 1: support[m-chunk c][p, o] = sum_i x[128c+p, i] dw[i, o]
        # xT tiles via PE transpose, grouped 4 per PSUM bank (2 m-chunks).
        support = singles.tile([P, MB, DOUT], F32R)
        for g in range(MB // 2):
            ptg = psum_t.tile([P, 4 * P], F32R, tag="pt")
            for k in range(2):
                c = 2 * g + k
                for ib in range(IB):
                    j = 2 * k + ib
                    nc.tensor.transpose(
                        ptg[:, j * P : (j + 1) * P],
                        x_sb[:, c, ts(ib, P)],
                        ident,
                    )
            if g % 2 == 0:
                atg = at_dve.tile([P, 4 * P], F32R, tag="at_dve")
                nc.vector.tensor_copy(atg, ptg)
            else:
                atg = at_act.tile([P, 4 * P], F32R, tag="at_act")
                nc.scalar.copy(atg, ptg)
            for k in range(2):
                c = 2 * g + k
                spsum = psum_acc.tile([P, DOUT], F32, tag="acc")
                for ib in range(IB):
                    j = 2 * k + ib
                    nc.tensor.matmul(
                        spsum,
                        lhsT=atg[:, j * P : (j + 1) * P],
                        rhs=dw_sb[:, ib, :],
                        start=(ib == 0),
                        stop=(ib == IB - 1),
                    )
                if c % 2 == 0:
                    nc.vector.tensor_copy(support[:, c, :], spsum)
                else:
                    nc.scalar.copy(support[:, c, :], spsum)

        # stage 2: out[n-block nb] = relu(adj[nb rows] @ support + bias).
        # Software-pipelined emission: the transpose group for global group
        # G is emitted BEFORE the matmuls of group G-1, so the PE's static
        # instruction order interleaves them and the PSUM->SBUF copy latency
        # of group G hides behind the matmuls of G-1 (instead of the PE
        # head-of-line blocking on its own just-issued transpose's copy).
        GPN = MB // 4  # transpose groups per n-block
        arows = {}
        opsums = {}

        def emit_tgroup(G):
            nb, g = divmod(G, GPN)
            if g == 0:
                arow = arow_pool.tile([P, N], F32R, tag="arow", name=f"arow{nb}")
                nc.sync.dma_start(out=arow, in_=adj_d[ts(nb, P), :])
                arows[nb] = arow
                opsums[nb] = psum_acc.tile(
                    [P, DOUT], F32, tag="acc", name=f"opsum{nb}"
                )
            arow = arows[nb]
            ptg = psum_t.tile([P, 4 * P], F32R, tag="pt")
            for k in range(4):
                mb = 4 * g + k
                nc.tensor.transpose(
                    ptg[:, k * P : (k + 1) * P], arow[:, ts(mb, P)], ident
                )
            if g % 2 == 0:
                atg = at_dve.tile([P, 4 * P], F32R, tag="at_dve")
                nc.vector.tensor_copy(atg, ptg)
            else:
                atg = at_act.tile([P, 4 * P], F32R, tag="at_act")
                nc.scalar.copy(atg, ptg)
            return atg

        def emit_mgroup(G, atg):
            nb, g = divmod(G, GPN)
            opsum = opsums[nb]
            for k in range(4):
                mb = 4 * g + k
                nc.tensor.matmul(
                    opsum,
                    lhsT=atg[:, k * P : (k + 1) * P],
                    rhs=support[:, mb, :],
                    start=(mb == 0),
                    stop=False,
                )
            if g == GPN - 1:
                nc.tensor.matmul(
                    opsum, lhsT=ones, rhs=bias_sb, start=False, stop=True
                )
                out_sb = out_pool.tile([P, DOUT], F32)
                nc.scalar.activation(
                    out_sb, opsum, mybir.ActivationFunctionType.Relu
                )
                # out stores ride the ACT HWDGE queue so they never
                # head-of-line block the SP queue's adj row prefetch stream.
                nc.scalar.dma_start(out=out_d[ts(nb, P), :], in_=out_sb)

        NGROUPS = NB * GPN
        LOOKAHEAD = 4
        pending = [emit_tgroup(G) for G in range(LOOKAHEAD)]
        for G in range(LOOKAHEAD, NGROUPS):
            nxt = emit_tgroup(G)
            emit_mgroup(G - LOOKAHEAD, pending.pop(0))
            pending.append(nxt)
        for i, atg in enumerate(pending):
            emit_mgroup(NGROUPS - LOOKAHEAD + i, atg)

    nc.compile()
    return nc


def _get_nc():
    global _NC
    if _NC is None:
        _NC = _build_nc()
    return _NC


def kernel(x, adj, weight, bias, _trace=False):
    from concourse import bass_utils

    x = np.ascontiguousarray(np.asarray(x, dtype=np.float32))
    adj = np.ascontiguousarray(np.asarray(adj, dtype=np.float32))
    weight = np.ascontiguousarray(np.asarray(weight, dtype=np.float32))
    bias = np.ascontiguousarray(np.asarray(bias, dtype=np.float32))

    nc = _get_nc()
    in_maps = [
        {"x": x[b], "adj": adj[b], "weight": weight, "bias": bias}
        for b in range(B)
    ]
    res = bass_utils.run_bass_kernel_spmd(
        nc, in_maps, core_ids=list(range(B)), trace=_trace
    )
    out = np.stack([r["out"] for r in res.results], axis=0)
    if _trace:
        return out, res
    return out
